# revision 13
# baseline (speedup 1.0000x reference)
"""Trainium2 Bass kernel for nn_InteractionModule (GNN message passing).

v2 strategy (8 NeuronCores, SPMD, no collectives):
 - Nodes sharded 8 x 6250 by dst; edges assigned to the core owning dst,
   sorted by dst chunk, padded to 128-edge subtiles equalized across cores.
 - Phase A: every core computes the spE table spE[n] = ssp(W_diff ssp(x_n) + b)
   (f16, post-activation) over all N nodes in 25 windows of 2048, with
   activation instructions batched into same-function runs (4 windows per
   group) to avoid act-table reload flapping.  Matmuls in fp32r (1 cyc/col).
   Rows formed with dma_start_transpose; table stored block-cyclic
   (slot = (n%128)*NB + n//128) so each window writes 128 x 4KB descriptors.
 - Phase 2: batched indirect gathers (16 subtiles = 2048 descriptors per
   Pool instruction), gate = ea @ G_w.T per subtile on PE (f16), msg =
   spE_gathered * gate on DVE, segment-sum via one-hot matmul into PSUM
   (4-chunk groups), finish ssp batched over 4 chunks.
 - Phase 3: residual stack in transposed [f, node] layout over the whole
   shard (6656 cols) with batched activations; outputs via DMA transpose.
"""

import numpy as np

N, E, F, K, R = 50000, 600000, 128, 64, 3
NC_ = 8
NSH = N // NC_            # 6250 nodes per core
CHUNK = 256               # scatter window (one-hot width)
NCHUNK = (NSH + CHUNK - 1) // CHUNK   # 25
WINA = 2048               # phase A window
NWA = 25                  # windows over padded node range
NPAD = NWA * WINA         # 51200 padded nodes
NB = NPAD // 128          # 400 blocks per partition
GRPA = 4                  # phase A windows per act-batch group
HALF = 32768              # dma_gather int16 index limit
FCH = 4                   # chunks per finish group
NSHP = 6656               # padded shard width (13*512)
NBLK = 49                 # output blocks (49*128 = 6272 >= 6250)
LOG2 = float(np.log(2.0))

_cache = {}


def _prep(x, edge_index, edge_attr):
    """Host-side sharding: per-core edge arrays + structure lists.

    Within each (core, chunk), lanes are arranged [low-slot edges | pad |
    high-slot edges | pad] with the low/high boundary at the core-uniform
    subtile LB[c], so the spE gather is two dma_gather calls per chunk
    (int16 indices < 32768 each, all entries valid; dummy 0 in padding).
    """
    src = np.asarray(edge_index[0], dtype=np.int64)
    dst = np.asarray(edge_index[1], dtype=np.int64)
    core = dst // NSH
    dstl = dst - core * NSH
    ea = np.asarray(edge_attr, dtype=np.float32)

    chunk = dstl // CHUNK
    slot = (src % 128) * NB + src // 128      # block-cyclic table slot
    is_high = (slot >= HALF).astype(np.int64)
    key = (core * NCHUNK + chunk) * 2 + is_high
    order = np.argsort(key, kind="stable")
    counts = np.bincount(key[order], minlength=NC_ * NCHUNK * 2)
    counts = counts.reshape(NC_, NCHUNK, 2)
    stl = (counts[:, :, 0] + 127) // 128      # low subtiles per (core, chunk)
    sth = (counts[:, :, 1] + 127) // 128
    LB = stl.max(axis=0)
    HBm = sth.max(axis=0)
    st_max = ((LB + HBm + 7) // 8) * 8        # chunk subtiles, mult of 8
    S = int(st_max.sum())
    G = S // 4
    bases = np.zeros(NCHUNK + 1, np.int64)
    np.cumsum(st_max, out=bases[1:])

    chunk_of_q = np.repeat(np.arange(NCHUNK), st_max)
    firsts = np.zeros(S, bool)
    lasts = np.zeros(S, bool)
    for c in range(NCHUNK):
        firsts[bases[c]] = True
        lasts[bases[c + 1] - 1] = True

    idx_a = np.zeros((NC_, S * 128), np.int32)
    dstf_a = np.full((NC_, S * 128), -1.0, np.float32)
    ea_a = np.zeros((NC_, S * 128, K), np.float16)
    cum = np.zeros(NC_ * NCHUNK * 2 + 1, np.int64)
    np.cumsum(counts.ravel(), out=cum[1:])
    for c in range(NC_):
        for ch in range(NCHUNK):
            for hi in range(2):
                k0 = cum[(c * NCHUNK + ch) * 2 + hi]
                n_e = counts[c, ch, hi]
                sl = order[k0 : k0 + n_e]
                pos = (bases[ch] + (LB[ch] if hi else 0)) * 128
                idx_a[c, pos : pos + n_e] = slot[sl].astype(np.int32)
                dstf_a[c, pos : pos + n_e] = (dstl[sl] - ch * CHUNK).astype(np.float32)
                ea_a[c, pos : pos + n_e] = ea[sl].astype(np.float16)

    src_d = idx_a.reshape(NC_, S, 128).transpose(0, 2, 1).copy()

    dstf_d = dstf_a.reshape(NC_, S, 128).transpose(0, 2, 1).copy()
    eaT = ea_a.reshape(NC_, G, 512, K).transpose(0, 1, 3, 2)  # [NC, G, K, 512]
    Gp = (G + 1) // 2
    ea_d = np.zeros((NC_, 128, Gp * 512), np.float16)
    ev = eaT[:, 0::2]
    ea_d[:, :K, : ev.shape[1] * 512] = ev.transpose(0, 2, 1, 3).reshape(NC_, K, -1)
    od = eaT[:, 1::2]
    ea_d[:, K : 2 * K, : od.shape[1] * 512] = od.transpose(0, 2, 1, 3).reshape(NC_, K, -1)

    meta = dict(S=S, G=G, Gp=Gp,
                chunk_of_q=chunk_of_q.tolist(),
                firsts=firsts.tolist(), lasts=lasts.tolist(),
                bases=bases.tolist(), LB=LB.tolist(), st_max=st_max.tolist())
    return src_d, dstf_d, ea_d, meta


def _build(nc, meta):
    import contextlib
    import concourse.bass as bass
    import concourse.mybir as mybir
    import concourse.tile as tile

    F32, F16, BF16, I32 = (mybir.dt.float32, mybir.dt.float16,
                           mybir.dt.bfloat16, mybir.dt.int32)
    F32R = mybir.dt.float32r
    AF, ALU = mybir.ActivationFunctionType, mybir.AluOpType
    S, G, Gp = meta["S"], meta["G"], meta["Gp"]
    cq, firsts, lasts = meta["chunk_of_q"], meta["firsts"], meta["lasts"]
    bases, LBs, sts = meta["bases"], meta["LB"], meta["st_max"]
    STMX = max(sts)

    xT = nc.dram_tensor("xT", [F, NPAD], F16, kind="ExternalInput").ap()
    xTo = nc.dram_tensor("xTo", [F, NSHP], F16, kind="ExternalInput").ap()
    wpack = nc.dram_tensor("wpack", [F, 9 * F], F32, kind="ExternalInput").ap()
    wpackb = nc.dram_tensor("wpackb", [F, 9 * F], BF16, kind="ExternalInput").ap()
    bpack = nc.dram_tensor("bpack", [F, 16], F32, kind="ExternalInput").ap()
    gw2 = nc.dram_tensor("gw2", [128, 128], F16, kind="ExternalInput").ap()
    wdiffb_in = nc.dram_tensor("wdiffb_in", [F, F], BF16, kind="ExternalInput").ap()
    iota_in = nc.dram_tensor("iota_in", [128, CHUNK], F16, kind="ExternalInput").ap()
    src_in = nc.dram_tensor("src_in", [128, S], I32, kind="ExternalInput").ap()
    dstf_in = nc.dram_tensor("dstf_in", [128, S], F32, kind="ExternalInput").ap()
    ea_in = nc.dram_tensor("ea_in", [128, Gp * 512], F16, kind="ExternalInput").ap()
    out0 = nc.dram_tensor("out0", [128, NBLK * 128], F16, kind="ExternalOutput").ap()
    out1 = nc.dram_tensor("out1", [128, NBLK * 128], F16, kind="ExternalOutput").ap()

    with tile.TileContext(nc) as tc, contextlib.ExitStack() as ctx:
        const = ctx.enter_context(tc.tile_pool(name="const", bufs=1))
        big = ctx.enter_context(tc.tile_pool(name="big", bufs=1))
        dram = ctx.enter_context(tc.tile_pool(name="dram", bufs=1, space="DRAM"))

        spE = dram.tile([NPAD, F], F16)
        spE_b = spE[:].rearrange("(p b) f -> p b f", p=128)  # slot = p*NB + b

        wp = const.tile([F, 9 * F], F32)
        nc.sync.dma_start(wp[:], wpack)
        wpb = const.tile([F, 9 * F], BF16)
        nc.sync.dma_start(wpb[:], wpackb)
        bp = const.tile([F, 16], F32)
        nc.sync.dma_start(bp[:], bpack)
        gw = const.tile([128, 128], F16)
        nc.sync.dma_start(gw[:], gw2)
        wdb = const.tile([F, F], BF16)
        nc.sync.dma_start(wdb[:], wdiffb_in)
        iota = const.tile([128, CHUNK], F16)
        nc.sync.dma_start(iota[:], iota_in)
        srcs = const.tile([128, S], I32)
        nc.sync.dma_start(srcs[:], src_in)
        dstf = const.tile([128, S], F32)
        nc.sync.dma_start(dstf[:], dstf_in)
        half = const.tile([128, 1], F32)
        nc.gpsimd.memset(half[:], 0.5)

        W_sameT = wp[:, F : 2 * F]
        W1T = [wpb[:, (2 + i) * F : (3 + i) * F] for i in range(3)]
        W2T = [wpb[:, (5 + i) * F : (6 + i) * F] for i in range(3)]
        W_lastT = wpb[:, 8 * F : 9 * F]
        b_diff = bp[:, 0:1]
        b_same = bp[:, 1:2]
        b1 = [bp[:, 2 + i : 3 + i] for i in range(3)]
        b2 = [bp[:, 5 + i : 6 + i] for i in range(3)]
        b_last = bp[:, 8:9]
        uT = bp[:, 9:10]

        z_sT = big.tile([128, NSHP], F32)
        xuT = big.tile([128, NSHP], F16)
        nc.gpsimd.memset(z_sT[:, NSH:NSHP], 0.0)

        # ---- phase A: replicated spE table over all padded nodes ----
        with tc.tile_pool(name="pxt", bufs=4) as pxt, \
             tc.tile_pool(name="pex", bufs=4) as pex, \
             tc.tile_pool(name="pxa", bufs=4) as pxa, \
             tc.tile_pool(name="pye", bufs=4) as pye, \
             tc.tile_pool(name="psp", bufs=4) as psp, \
             tc.tile_pool(name="prow", bufs=4) as prow, \
             tc.tile_pool(name="psA", bufs=4, space="PSUM") as psA:
            for g0 in range(0, NWA, GRPA):
                grp = range(g0, min(g0 + GRPA, NWA))
                xts, exs, xas, yes, sps, rows = {}, {}, {}, {}, {}, {}
                for w in grp:
                    xts[w] = pxt.tile([128, WINA], F16, tag="xt", name="xt")
                    nc.sync.dma_start(xts[w][:], xT[:, w * WINA : (w + 1) * WINA])
                for w in grp:
                    exs[w] = pex.tile([128, WINA], F16, tag="ex", name="ex")
                    nc.scalar.activation(exs[w][:], xts[w][:], AF.Exp)
                for w in grp:
                    xas[w] = pxa.tile([128, WINA], BF16, tag="xa", name="xa")
                    nc.scalar.activation(xas[w][:], exs[w][:], AF.Ln,
                                         bias=half[:, 0:1], scale=0.5)
                for w in grp:
                    yes[w] = pye.tile([128, WINA], F16, tag="ye", name="ye")
                    for j in range(WINA // 512):
                        ps = psA.tile([128, 512], F32, tag="mm")
                        nc.tensor.matmul(ps[:], wdb[:],
                                         xas[w][:, 512 * j : 512 * (j + 1)],
                                         start=True, stop=True, skip_group_check=True)
                        nc.scalar.activation(yes[w][:, 512 * j : 512 * (j + 1)],
                                             ps[:], AF.Exp, bias=b_diff)
                for w in grp:
                    sps[w] = psp.tile([128, WINA], F16, tag="sp", name="sp")
                    nc.scalar.activation(sps[w][:], yes[w][:], AF.Ln,
                                         bias=half[:, 0:1], scale=0.5)
                for w in grp:
                    rows[w] = prow.tile([128, WINA // 128, 128], F16, tag="row", name="row")
                    nc.sync.dma_start_transpose(rows[w][:], sps[w][:])
                nb_w = WINA // 128  # 16 blocks per window
                for w in grp:
                    nc.sync.dma_start(spE_b[:, w * nb_w : (w + 1) * nb_w, :],
                                      rows[w][:])

        # ---- phase 1b: own-shard z_same, xu ----
        with tc.tile_pool(name="p1b", bufs=1) as p1b, \
             tc.tile_pool(name="ps1b", bufs=4, space="PSUM") as ps1b:
            xto = p1b.tile([128, NSHP], F16)
            nc.sync.dma_start(xto[:], xTo)
            nc.vector.tensor_scalar_mul(xuT[:], xto[:], uT)
            ext = p1b.tile([128, NSHP], F16)
            nc.scalar.activation(ext[:], xto[:], AF.Exp)
            xa1 = p1b.tile([128, NSHP], F32)
            nc.scalar.activation(xa1[:], ext[:], AF.Ln, bias=half[:, 0:1], scale=0.5)
            for j in range(NSHP // 512):
                ps = ps1b.tile([128, 512], F32, tag="mm")
                nc.tensor.matmul(ps[:], W_sameT,
                                 xa1[:, 512 * j : 512 * (j + 1)],
                                 start=True, stop=True, skip_group_check=True)
                nc.vector.tensor_scalar_add(z_sT[:, 512 * j : 512 * (j + 1)],
                                            ps[:], b_same)

        # ---- phase 2: edges ----
        nfin = (NCHUNK + FCH - 1) // FCH
        lastq_of_grp = {}
        for q in range(S):
            if lasts[q]:
                c = cq[q]
                if c % FCH == FCH - 1 or c == NCHUNK - 1:
                    lastq_of_grp[q] = c // FCH

        with tc.tile_pool(name="pyg", bufs=3) as pyg, \
             tc.tile_pool(name="peat", bufs=3) as peat, \
             tc.tile_pool(name="pmsg", bufs=3) as pmsg, \
             tc.tile_pool(name="poh", bufs=6) as poh, \
             tc.tile_pool(name="pfin", bufs=2) as pfin, \
             tc.tile_pool(name="psG", bufs=2, space="PSUM") as psG, \
             tc.tile_pool(name="psAg", bufs=2, space="PSUM") as psAg:

            oh_tiles = {}

            def build_oh(h):
                oh = poh.tile([128, 8, CHUNK], F16, tag="oh", name="oh")
                q0 = 8 * h
                for t in range(8):
                    nc.vector.tensor_scalar(oh[:, t, :], iota[:],
                                            dstf[:, q0 + t : q0 + t + 1], 0.0,
                                            ALU.subtract, ALU.is_equal)
                oh_tiles[h] = oh

            NHB = S // 8
            for h in range(min(4, NHB)):
                build_oh(h)

            lastq_of_grp = {}
            for q in range(S):
                if lasts[q]:
                    c = cq[q]
                    if c % FCH == FCH - 1 or c == NCHUNK - 1:
                        lastq_of_grp[q] = c // FCH

            aggr = None
            for c in range(NCHUNK):
                base, st, LB = bases[c], sts[c], LBs[c]
                HB = st - LB
                ygc = pyg.tile([128, STMX, F], F16, tag="yg", name="ygc")
                for sq in range(st):
                    q = base + sq
                    nc.gpsimd.indirect_dma_start(
                        out=ygc[:, sq, :], out_offset=None, in_=spE[:],
                        in_offset=bass.IndirectOffsetOnAxis(
                            ap=srcs[:, q : q + 1], axis=0))
                for pr in range(st // 8):
                    h = base // 8 + pr
                    q0 = base + pr * 8
                    eat = peat.tile([128, 512], F16, tag="eat")
                    nc.sync.dma_start(eat[:], ea_in[:, 512 * h : 512 * (h + 1)])
                    gps = psG.tile([128, 8, F], F32, tag="gate")
                    for t in range(4):
                        nc.tensor.matmul(gps[:, t, :],
                                         eat[0:64, 128 * t : 128 * (t + 1)],
                                         gw[0:64, :], start=True, stop=True,
                                         skip_group_check=True)
                    for t in range(4):
                        nc.tensor.matmul(gps[:, 4 + t, :],
                                         eat[64:128, 128 * t : 128 * (t + 1)],
                                         gw[64:128, :], start=True, stop=True,
                                         skip_group_check=True)
                    if h + 2 < NHB:
                        build_oh(h + 2)
                    msg = pmsg.tile([128, 8, F], F16, tag="msg")
                    nc.vector.tensor_tensor(
                        msg[:].rearrange("p a b -> p (a b)"),
                        ygc[:, 8 * pr : 8 * (pr + 1), :].rearrange("p a b -> p (a b)"),
                        gps[:].rearrange("p a b -> p (a b)"), ALU.mult)
                    oh = oh_tiles.pop(h)
                    for t in range(8):
                        q = q0 + t
                        if firsts[q] and c % FCH == 0:
                            aggr = psAg.tile([128, FCH * CHUNK], F32, tag="aggr",
                                             name="aggr")
                        cw = (c % FCH) * CHUNK
                        nc.tensor.matmul(aggr[:, cw : cw + CHUNK], msg[:, t, :],
                                         oh[:, t, :], start=bool(firsts[q]),
                                         stop=bool(lasts[q]), skip_group_check=True)
                        if q in lastq_of_grp:
                            cg = lastq_of_grp[q]
                            c0 = cg * FCH
                            ncc = min(FCH, NCHUNK - c0) * CHUNK
                            s0 = c0 * CHUNK
                            ef = pfin.tile([128, FCH * CHUNK], BF16, tag="ef")
                            nc.scalar.activation(ef[:, :ncc], z_sT[:, s0 : s0 + ncc],
                                                 AF.Exp)
                            sf = pfin.tile([128, FCH * CHUNK], BF16, tag="sf")
                            nc.scalar.activation(sf[:, :ncc], ef[:, :ncc], AF.Ln,
                                                 bias=half[:, 0:1], scale=0.5)
                            nc.vector.tensor_tensor(z_sT[:, s0 : s0 + ncc],
                                                    sf[:, :ncc], aggr[:, :ncc],
                                                    ALU.add)

        # ---- phase 3: residual stack + outputs ----
        with tc.tile_pool(name="pt16", bufs=2) as pt16, \
             tc.tile_pool(name="ps32", bufs=2) as ps32, \
             tc.tile_pool(name="po16", bufs=1) as po16, \
             tc.tile_pool(name="prw3", bufs=1) as prw3, \
             tc.tile_pool(name="ps3", bufs=4, space="PSUM") as ps3:
            # out1 = msged_x rows
            o1c = po16.tile([128, NSHP], F16, tag="o16")
            nc.vector.tensor_copy(o1c[:, : NBLK * 128], z_sT[:, : NBLK * 128])
            o1r = prw3.tile([128, NBLK, 128], F16, tag="rw")
            nc.sync.dma_start_transpose(o1r[:], o1c[:, : NBLK * 128])
            nc.sync.dma_start(out1, o1r[:].rearrange("p a b -> p (a b)"))

            cur = z_sT
            for i in range(R):
                e1 = pt16.tile([128, NSHP], BF16, tag="e16")
                nc.scalar.activation(e1[:], cur[:], AF.Exp)
                s1 = ps32.tile([128, NSHP], BF16, tag="s32")
                nc.scalar.activation(s1[:], e1[:], AF.Ln, bias=half[:, 0:1], scale=0.5)
                e2 = pt16.tile([128, NSHP], BF16, tag="e16")
                for j in range(NSHP // 512):
                    ps = ps3.tile([128, 512], F32, tag="mm")
                    nc.tensor.matmul(ps[:], W1T[i],
                                     s1[:, 512 * j : 512 * (j + 1)],
                                     start=True, stop=True, skip_group_check=True)
                    nc.scalar.activation(e2[:, 512 * j : 512 * (j + 1)], ps[:],
                                         AF.Exp, bias=b1[i])
                s2 = ps32.tile([128, NSHP], BF16, tag="s32")
                nc.scalar.activation(s2[:], e2[:], AF.Ln, bias=half[:, 0:1], scale=0.5)
                for j in range(NSHP // 512):
                    ps = ps3.tile([128, 512], F32, tag="mm")
                    nc.tensor.matmul(ps[:], W2T[i],
                                     s2[:, 512 * j : 512 * (j + 1)],
                                     start=True, stop=True, skip_group_check=True)
                    nc.vector.scalar_tensor_tensor(
                        cur[:, 512 * j : 512 * (j + 1)], ps[:], b2[i],
                        cur[:, 512 * j : 512 * (j + 1)], ALU.add, ALU.add)
            ev = pt16.tile([128, NSHP], BF16, tag="e16")
            nc.scalar.activation(ev[:], cur[:], AF.Exp)
            sv = ps32.tile([128, NSHP], BF16, tag="s32")
            nc.scalar.activation(sv[:], ev[:], AF.Ln, bias=half[:, 0:1], scale=0.5)
            o0t = po16.tile([128, NSHP], F16, tag="o16")
            for j in range(NSHP // 512):
                ps = ps3.tile([128, 512], F32, tag="mm")
                nc.tensor.matmul(ps[:], W_lastT,
                                 sv[:, 512 * j : 512 * (j + 1)],
                                 start=True, stop=True, skip_group_check=True)
                nc.vector.scalar_tensor_tensor(
                    o0t[:, 512 * j : 512 * (j + 1)], ps[:], b_last,
                    xuT[:, 512 * j : 512 * (j + 1)], ALU.add, ALU.add)
            o0r = prw3.tile([128, NBLK, 128], F16, tag="rw")
            nc.sync.dma_start_transpose(o0r[:], o0t[:, : NBLK * 128])
            nc.sync.dma_start(out0, o0r[:].rearrange("p a b -> p (a b)"))
    return nc


def kernel(**inputs):
    import concourse.bacc as bacc
    from concourse import bass_utils

    x = np.asarray(inputs["x"], np.float32)
    src_d, dstf_d, ea_d, meta = _prep(x, inputs["edge_index"], inputs["edge_attr"])

    key = (meta["S"], meta["G"])
    if key not in _cache:
        nc = bacc.Bacc("TRN2", target_bir_lowering=False, debug=False,
                       enable_asserts=False, num_devices=NC_,
                       dynamic_dma_scratch_size=32768)
        _build(nc, meta)
        nc.compile()
        _cache[key] = nc
    nc = _cache[key]

    wpack = np.concatenate(
        [np.asarray(inputs[k], np.float32).T.copy() for k in ["W_diff", "W_same"]]
        + [np.asarray(inputs["res_W1"][i], np.float32).T.copy() for i in range(3)]
        + [np.asarray(inputs["res_W2"][i], np.float32).T.copy() for i in range(3)]
        + [np.asarray(inputs["W_last"], np.float32).T.copy()], axis=1)
    bpack = np.zeros((F, 16), np.float32)
    bpack[:, 0] = np.asarray(inputs["b_diff"], np.float32)
    bpack[:, 1] = np.asarray(inputs["b_same"], np.float32)
    for i in range(3):
        bpack[:, 2 + i] = np.asarray(inputs["res_b1"][i], np.float32)
        bpack[:, 5 + i] = np.asarray(inputs["res_b2"][i], np.float32)
    bpack[:, 8] = np.asarray(inputs["b_last"], np.float32)
    bpack[:, 9] = np.asarray(inputs["u"], np.float32)[0]
    G_w = np.asarray(inputs["G_w"], np.float32)
    gw2 = np.zeros((128, 128), np.float16)
    gw2[:K] = G_w.T.astype(np.float16)
    gw2[64 : 64 + K] = G_w.T.astype(np.float16)
    iota = np.broadcast_to(np.arange(CHUNK, dtype=np.float16), (128, CHUNK)).copy()
    import ml_dtypes
    wdiffb = np.asarray(inputs["W_diff"], np.float32).T.copy().astype(ml_dtypes.bfloat16)
    wpackb = wpack.astype(ml_dtypes.bfloat16)

    xT = np.zeros((F, NPAD), np.float16)
    xT[:, :N] = x.T
    in_maps = []
    for c in range(NC_):
        xTo = np.zeros((F, NSHP), np.float16)
        xTo[:, :NSH] = x.T[:, c * NSH : (c + 1) * NSH]
        in_maps.append(dict(
            xT=xT, xTo=xTo, wpack=wpack, wpackb=wpackb, bpack=bpack, gw2=gw2, iota_in=iota,
            wdiffb_in=wdiffb, src_in=src_d[c], dstf_in=dstf_d[c], ea_in=ea_d[c],
        ))
    res = bass_utils.run_bass_kernel_spmd(nc, in_maps, core_ids=list(range(NC_)))

    def unblk(a):
        return (a.reshape(128, NBLK, 128).transpose(1, 0, 2)
                 .reshape(NBLK * 128, F)[:NSH].astype(np.float32))

    o0 = np.concatenate([unblk(res.results[c]["out0"]) for c in range(NC_)], axis=0)
    o1 = np.concatenate([unblk(res.results[c]["out1"]) for c in range(NC_)], axis=0)
    return (o0, o1)


# revision 14
# speedup vs baseline: 1.0826x; 1.0826x over previous
"""Trainium2 Bass kernel for nn_InteractionModule (GNN message passing).

v2 strategy (8 NeuronCores, SPMD, no collectives):
 - Nodes sharded 8 x 6250 by dst; edges assigned to the core owning dst,
   sorted by dst chunk, padded to 128-edge subtiles equalized across cores.
 - Phase A: every core computes the spE table spE[n] = ssp(W_diff ssp(x_n) + b)
   (f16, post-activation) over all N nodes in 25 windows of 2048, with
   activation instructions batched into same-function runs (4 windows per
   group) to avoid act-table reload flapping.  Matmuls in fp32r (1 cyc/col).
   Rows formed with dma_start_transpose; table stored block-cyclic
   (slot = (n%128)*NB + n//128) so each window writes 128 x 4KB descriptors.
 - Phase 2: batched indirect gathers (16 subtiles = 2048 descriptors per
   Pool instruction), gate = ea @ G_w.T per subtile on PE (f16), msg =
   spE_gathered * gate on DVE, segment-sum via one-hot matmul into PSUM
   (4-chunk groups), finish ssp batched over 4 chunks.
 - Phase 3: residual stack in transposed [f, node] layout over the whole
   shard (6656 cols) with batched activations; outputs via DMA transpose.
"""

import numpy as np

N, E, F, K, R = 50000, 600000, 128, 64, 3
NC_ = 8
NSH = N // NC_            # 6250 nodes per core
CHUNK = 256               # scatter window (one-hot width)
NCHUNK = (NSH + CHUNK - 1) // CHUNK   # 25
WINA = 2048               # phase A window
NWA = 25                  # windows over padded node range
NPAD = NWA * WINA         # 51200 padded nodes
NB = NPAD // 128          # 400 blocks per partition
GRPA = 4                  # phase A windows per act-batch group
HALF = 32768              # dma_gather int16 index limit
FCH = 4                   # chunks per finish group
NSHP = 6656               # padded shard width (13*512)
NBLK = 49                 # output blocks (49*128 = 6272 >= 6250)
LOG2 = float(np.log(2.0))

_cache = {}


def _prep(x, edge_index, edge_attr):
    """Host-side sharding.

    Edges are binned by (dst core, dst chunk) and sorted by src within each
    bin, so successive subtiles of a chunk reference increasing table
    prefixes; each subtile's gather depends only on spE[0:limit_q], letting
    gathers overlap the table build.  Chunks are padded to supertile
    multiples (4 subtiles), equalized across cores.
    """
    src = np.asarray(edge_index[0], dtype=np.int64)
    dst = np.asarray(edge_index[1], dtype=np.int64)
    core = dst // NSH
    dstl = dst - core * NSH
    ea = np.asarray(edge_attr, dtype=np.float32)

    chunk = dstl // CHUNK
    key = (core * NCHUNK + chunk) * (N + 1) + src   # sort by bin then src
    order = np.argsort(key, kind="stable")
    counts = np.bincount(core * NCHUNK + chunk, minlength=NC_ * NCHUNK)
    counts = counts.reshape(NC_, NCHUNK)
    st = (counts + 127) // 128
    st_max = ((st.max(axis=0) + 3) // 4) * 4        # mult of 4 (supertiles)
    S = int(st_max.sum())
    if S % 8:                                       # pair alignment for ea
        st_max[-1] += 4
        S += 4
    G = S // 4
    bases = np.zeros(NCHUNK + 1, np.int64)
    np.cumsum(st_max, out=bases[1:])

    chunk_of_q = np.repeat(np.arange(NCHUNK), st_max)
    firsts = np.zeros(S, bool)
    lasts = np.zeros(S, bool)
    for c in range(NCHUNK):
        firsts[bases[c]] = True
        lasts[bases[c + 1] - 1] = True

    src_a = np.zeros((NC_, S * 128), np.int32)
    dstf_a = np.full((NC_, S * 128), -1.0, np.float32)
    ea_a = np.zeros((NC_, S * 128, K), np.float16)
    cum = np.zeros(NC_ * NCHUNK + 1, np.int64)
    np.cumsum(counts.ravel(), out=cum[1:])
    for c in range(NC_):
        for ch in range(NCHUNK):
            k0 = cum[c * NCHUNK + ch]
            n_e = counts[c, ch]
            sl = order[k0 : k0 + n_e]
            pos = bases[ch] * 128
            src_a[c, pos : pos + n_e] = src[sl]
            dstf_a[c, pos : pos + n_e] = (dstl[sl] - ch * CHUNK).astype(np.float32)
            ea_a[c, pos : pos + n_e] = ea[sl].astype(np.float16)

    # per-subtile table prefix needed (max over cores, window-aligned)
    smax = src_a.reshape(NC_, S, 128).max(axis=2).max(axis=0)  # [S]
    limit_q = np.minimum((smax // WINA + 1) * WINA, NPAD).astype(np.int64)

    src_d = src_a.reshape(NC_, S, 128).transpose(0, 2, 1).copy()
    dstf_d = dstf_a.reshape(NC_, S, 128).transpose(0, 2, 1).copy()
    eaT = ea_a.reshape(NC_, G, 512, K).transpose(0, 1, 3, 2)  # [NC, G, K, 512]
    Gp = (G + 1) // 2
    ea_d = np.zeros((NC_, 128, Gp * 512), np.float16)
    ev = eaT[:, 0::2]
    ea_d[:, :K, : ev.shape[1] * 512] = ev.transpose(0, 2, 1, 3).reshape(NC_, K, -1)
    od = eaT[:, 1::2]
    ea_d[:, K : 2 * K, : od.shape[1] * 512] = od.transpose(0, 2, 1, 3).reshape(NC_, K, -1)

    meta = dict(S=S, G=G, Gp=Gp,
                chunk_of_q=chunk_of_q.tolist(),
                firsts=firsts.tolist(), lasts=lasts.tolist(),
                limit_q=limit_q.tolist())
    return src_d, dstf_d, ea_d, meta


def _build(nc, meta):
    import contextlib
    import concourse.bass as bass
    import concourse.mybir as mybir
    import concourse.tile as tile

    F32, F16, BF16, I32 = (mybir.dt.float32, mybir.dt.float16,
                           mybir.dt.bfloat16, mybir.dt.int32)
    F32R = mybir.dt.float32r
    AF, ALU = mybir.ActivationFunctionType, mybir.AluOpType
    S, G, Gp = meta["S"], meta["G"], meta["Gp"]
    cq, firsts, lasts = meta["chunk_of_q"], meta["firsts"], meta["lasts"]
    limit_q = meta["limit_q"]

    xT = nc.dram_tensor("xT", [F, NPAD], F16, kind="ExternalInput").ap()
    xTo = nc.dram_tensor("xTo", [F, NSHP], F16, kind="ExternalInput").ap()
    wpack = nc.dram_tensor("wpack", [F, 9 * F], F32, kind="ExternalInput").ap()
    wpackb = nc.dram_tensor("wpackb", [F, 9 * F], BF16, kind="ExternalInput").ap()
    bpack = nc.dram_tensor("bpack", [F, 16], F32, kind="ExternalInput").ap()
    gw2 = nc.dram_tensor("gw2", [128, 128], F16, kind="ExternalInput").ap()
    wdiffb_in = nc.dram_tensor("wdiffb_in", [F, F], BF16, kind="ExternalInput").ap()
    iota_in = nc.dram_tensor("iota_in", [128, CHUNK], F16, kind="ExternalInput").ap()
    src_in = nc.dram_tensor("src_in", [128, S], I32, kind="ExternalInput").ap()
    dstf_in = nc.dram_tensor("dstf_in", [128, S], F32, kind="ExternalInput").ap()
    ea_in = nc.dram_tensor("ea_in", [128, Gp * 512], F16, kind="ExternalInput").ap()
    out0 = nc.dram_tensor("out0", [128, NBLK * 128], F16, kind="ExternalOutput").ap()
    out1 = nc.dram_tensor("out1", [128, NBLK * 128], F16, kind="ExternalOutput").ap()

    with tile.TileContext(nc) as tc, contextlib.ExitStack() as ctx:
        const = ctx.enter_context(tc.tile_pool(name="const", bufs=1))
        big = ctx.enter_context(tc.tile_pool(name="big", bufs=1))
        dram = ctx.enter_context(tc.tile_pool(name="dram", bufs=1, space="DRAM"))

        spE = dram.tile([NPAD, F], F16)
        spE_b = spE[:].rearrange("(b p) f -> p b f", p=128)  # row = 128*b + p

        wp = const.tile([F, 9 * F], F32)
        nc.sync.dma_start(wp[:], wpack)
        wpb = const.tile([F, 9 * F], BF16)
        nc.sync.dma_start(wpb[:], wpackb)
        bp = const.tile([F, 16], F32)
        nc.sync.dma_start(bp[:], bpack)
        gw = const.tile([128, 128], F16)
        nc.sync.dma_start(gw[:], gw2)
        wdb = const.tile([F, F], BF16)
        nc.sync.dma_start(wdb[:], wdiffb_in)
        iota = const.tile([128, CHUNK], F16)
        nc.sync.dma_start(iota[:], iota_in)
        srcs = const.tile([128, S], I32)
        nc.sync.dma_start(srcs[:], src_in)
        dstf = const.tile([128, S], F32)
        nc.sync.dma_start(dstf[:], dstf_in)
        half = const.tile([128, 1], F32)
        nc.gpsimd.memset(half[:], 0.5)

        W_sameT = wp[:, F : 2 * F]
        W1T = [wpb[:, (2 + i) * F : (3 + i) * F] for i in range(3)]
        W2T = [wpb[:, (5 + i) * F : (6 + i) * F] for i in range(3)]
        W_lastT = wpb[:, 8 * F : 9 * F]
        b_diff = bp[:, 0:1]
        b_same = bp[:, 1:2]
        b1 = [bp[:, 2 + i : 3 + i] for i in range(3)]
        b2 = [bp[:, 5 + i : 6 + i] for i in range(3)]
        b_last = bp[:, 8:9]
        uT = bp[:, 9:10]

        z_sT = big.tile([128, NSHP], F32)
        xuT = big.tile([128, NSHP], F16)
        nc.gpsimd.memset(z_sT[:, NSH:NSHP], 0.0)

        # ---- phase A: replicated spE table over all padded nodes ----
        with tc.tile_pool(name="pxt", bufs=4) as pxt, \
             tc.tile_pool(name="pex", bufs=4) as pex, \
             tc.tile_pool(name="pxa", bufs=4) as pxa, \
             tc.tile_pool(name="pye", bufs=4) as pye, \
             tc.tile_pool(name="psp", bufs=4) as psp, \
             tc.tile_pool(name="prow", bufs=4) as prow, \
             tc.tile_pool(name="psA", bufs=4, space="PSUM") as psA:
            for g0 in range(0, NWA, GRPA):
                grp = range(g0, min(g0 + GRPA, NWA))
                xts, exs, xas, yes, sps, rows = {}, {}, {}, {}, {}, {}
                for w in grp:
                    xts[w] = pxt.tile([128, WINA], F16, tag="xt", name="xt")
                    nc.sync.dma_start(xts[w][:], xT[:, w * WINA : (w + 1) * WINA])
                for w in grp:
                    exs[w] = pex.tile([128, WINA], F16, tag="ex", name="ex")
                    nc.scalar.activation(exs[w][:], xts[w][:], AF.Exp)
                for w in grp:
                    xas[w] = pxa.tile([128, WINA], BF16, tag="xa", name="xa")
                    nc.scalar.activation(xas[w][:], exs[w][:], AF.Ln,
                                         bias=half[:, 0:1], scale=0.5)
                for w in grp:
                    yes[w] = pye.tile([128, WINA], F16, tag="ye", name="ye")
                    for j in range(WINA // 512):
                        ps = psA.tile([128, 512], F32, tag="mm")
                        nc.tensor.matmul(ps[:], wdb[:],
                                         xas[w][:, 512 * j : 512 * (j + 1)],
                                         start=True, stop=True, skip_group_check=True)
                        nc.scalar.activation(yes[w][:, 512 * j : 512 * (j + 1)],
                                             ps[:], AF.Exp, bias=b_diff)
                for w in grp:
                    sps[w] = psp.tile([128, WINA], F16, tag="sp", name="sp")
                    nc.scalar.activation(sps[w][:], yes[w][:], AF.Ln,
                                         bias=half[:, 0:1], scale=0.5)
                for w in grp:
                    rows[w] = prow.tile([128, WINA // 128, 128], F16, tag="row", name="row")
                    nc.sync.dma_start_transpose(rows[w][:], sps[w][:])
                nb_w = WINA // 128  # 16 blocks per window
                for w in grp:
                    nc.sync.dma_start(spE_b[:, w * nb_w : (w + 1) * nb_w, :],
                                      rows[w][:])

        # ---- phase 1b: own-shard z_same, xu ----
        with tc.tile_pool(name="p1b", bufs=1) as p1b, \
             tc.tile_pool(name="ps1b", bufs=4, space="PSUM") as ps1b:
            xto = p1b.tile([128, NSHP], F16)
            nc.sync.dma_start(xto[:], xTo)
            nc.vector.tensor_scalar_mul(xuT[:], xto[:], uT)
            ext = p1b.tile([128, NSHP], F16)
            nc.scalar.activation(ext[:], xto[:], AF.Exp)
            xa1 = p1b.tile([128, NSHP], F32)
            nc.scalar.activation(xa1[:], ext[:], AF.Ln, bias=half[:, 0:1], scale=0.5)
            for j in range(NSHP // 512):
                ps = ps1b.tile([128, 512], F32, tag="mm")
                nc.tensor.matmul(ps[:], W_sameT,
                                 xa1[:, 512 * j : 512 * (j + 1)],
                                 start=True, stop=True, skip_group_check=True)
                nc.vector.tensor_scalar_add(z_sT[:, 512 * j : 512 * (j + 1)],
                                            ps[:], b_same)

        # ---- phase 2: edges ----
        nfin = (NCHUNK + FCH - 1) // FCH
        lastq_of_grp = {}
        for q in range(S):
            if lasts[q]:
                c = cq[q]
                if c % FCH == FCH - 1 or c == NCHUNK - 1:
                    lastq_of_grp[q] = c // FCH

        with tc.tile_pool(name="pyg", bufs=3) as pyg, \
             tc.tile_pool(name="peat", bufs=3) as peat, \
             tc.tile_pool(name="pmsg", bufs=3) as pmsg, \
             tc.tile_pool(name="poh", bufs=6) as poh, \
             tc.tile_pool(name="pfin", bufs=2) as pfin, \
             tc.tile_pool(name="psG", bufs=2, space="PSUM") as psG, \
             tc.tile_pool(name="psAg", bufs=2, space="PSUM") as psAg:

            oh_tiles = {}

            def build_oh(g):
                oh = poh.tile([128, 4, CHUNK], F16, tag="oh", name="oh")
                q0 = 4 * g
                for t in range(4):
                    nc.vector.tensor_scalar(oh[:, t, :], iota[:],
                                            dstf[:, q0 + t : q0 + t + 1], 0.0,
                                            ALU.subtract, ALU.is_equal)
                oh_tiles[g] = oh

            for g in range(min(2, G)):
                build_oh(g)

            lastq_of_grp = {}
            for q in range(S):
                if lasts[q]:
                    c = cq[q]
                    if c % FCH == FCH - 1 or c == NCHUNK - 1:
                        lastq_of_grp[q] = c // FCH

            aggr = None
            for g in range(G):
                q0 = 4 * g
                yg = pyg.tile([128, 4, F], F16, tag="yg", name="yg")
                for t in range(4):
                    q = q0 + t
                    nc.gpsimd.indirect_dma_start(
                        out=yg[:, t, :], out_offset=None,
                        in_=spE[0 : limit_q[q], :],
                        in_offset=bass.IndirectOffsetOnAxis(
                            ap=srcs[:, q : q + 1], axis=0))
                if g % 2 == 0:
                    eat = peat.tile([128, 512], F16, tag="eat")
                    nc.sync.dma_start(eat[:], ea_in[:, 256 * g : 256 * g + 512])
                rb = 64 * (g % 2)
                gps = psG.tile([128, 4, F], F32, tag="gate")
                for t in range(4):
                    nc.tensor.matmul(gps[:, t, :],
                                     eat[rb : rb + 64, 128 * t : 128 * (t + 1)],
                                     gw[rb : rb + 64, :], start=True, stop=True,
                                     skip_group_check=True)
                if g + 2 < G:
                    build_oh(g + 2)
                msg = pmsg.tile([128, 4, F], F16, tag="msg")
                nc.vector.tensor_tensor(
                    msg[:].rearrange("p a b -> p (a b)"),
                    yg[:].rearrange("p a b -> p (a b)"),
                    gps[:].rearrange("p a b -> p (a b)"), ALU.mult)
                oh = oh_tiles.pop(g)
                for t in range(4):
                    q = q0 + t
                    c = cq[q]
                    if firsts[q] and c % FCH == 0:
                        aggr = psAg.tile([128, FCH * CHUNK], F32, tag="aggr",
                                         name="aggr")
                    cw = (c % FCH) * CHUNK
                    nc.tensor.matmul(aggr[:, cw : cw + CHUNK], msg[:, t, :],
                                     oh[:, t, :], start=bool(firsts[q]),
                                     stop=bool(lasts[q]), skip_group_check=True)
                    if q in lastq_of_grp:
                        cg = lastq_of_grp[q]
                        c0 = cg * FCH
                        ncc = min(FCH, NCHUNK - c0) * CHUNK
                        s0 = c0 * CHUNK
                        ef = pfin.tile([128, FCH * CHUNK], BF16, tag="ef")
                        nc.scalar.activation(ef[:, :ncc], z_sT[:, s0 : s0 + ncc],
                                             AF.Exp)
                        sf = pfin.tile([128, FCH * CHUNK], BF16, tag="sf")
                        nc.scalar.activation(sf[:, :ncc], ef[:, :ncc], AF.Ln,
                                             bias=half[:, 0:1], scale=0.5)
                        nc.vector.tensor_tensor(z_sT[:, s0 : s0 + ncc],
                                                sf[:, :ncc], aggr[:, :ncc],
                                                ALU.add)

        # ---- phase 3: residual stack + outputs ----
        with tc.tile_pool(name="pt16", bufs=2) as pt16, \
             tc.tile_pool(name="ps32", bufs=2) as ps32, \
             tc.tile_pool(name="po16", bufs=1) as po16, \
             tc.tile_pool(name="prw3", bufs=1) as prw3, \
             tc.tile_pool(name="ps3", bufs=4, space="PSUM") as ps3:
            # out1 = msged_x rows
            o1c = po16.tile([128, NSHP], F16, tag="o16")
            nc.vector.tensor_copy(o1c[:, : NBLK * 128], z_sT[:, : NBLK * 128])
            o1r = prw3.tile([128, NBLK, 128], F16, tag="rw")
            nc.sync.dma_start_transpose(o1r[:], o1c[:, : NBLK * 128])
            nc.sync.dma_start(out1, o1r[:].rearrange("p a b -> p (a b)"))

            cur = z_sT
            for i in range(R):
                e1 = pt16.tile([128, NSHP], BF16, tag="e16")
                nc.scalar.activation(e1[:], cur[:], AF.Exp)
                s1 = ps32.tile([128, NSHP], BF16, tag="s32")
                nc.scalar.activation(s1[:], e1[:], AF.Ln, bias=half[:, 0:1], scale=0.5)
                e2 = pt16.tile([128, NSHP], BF16, tag="e16")
                for j in range(NSHP // 512):
                    ps = ps3.tile([128, 512], F32, tag="mm")
                    nc.tensor.matmul(ps[:], W1T[i],
                                     s1[:, 512 * j : 512 * (j + 1)],
                                     start=True, stop=True, skip_group_check=True)
                    nc.scalar.activation(e2[:, 512 * j : 512 * (j + 1)], ps[:],
                                         AF.Exp, bias=b1[i])
                s2 = ps32.tile([128, NSHP], BF16, tag="s32")
                nc.scalar.activation(s2[:], e2[:], AF.Ln, bias=half[:, 0:1], scale=0.5)
                for j in range(NSHP // 512):
                    ps = ps3.tile([128, 512], F32, tag="mm")
                    nc.tensor.matmul(ps[:], W2T[i],
                                     s2[:, 512 * j : 512 * (j + 1)],
                                     start=True, stop=True, skip_group_check=True)
                    nc.vector.scalar_tensor_tensor(
                        cur[:, 512 * j : 512 * (j + 1)], ps[:], b2[i],
                        cur[:, 512 * j : 512 * (j + 1)], ALU.add, ALU.add)
            ev = pt16.tile([128, NSHP], BF16, tag="e16")
            nc.scalar.activation(ev[:], cur[:], AF.Exp)
            sv = ps32.tile([128, NSHP], BF16, tag="s32")
            nc.scalar.activation(sv[:], ev[:], AF.Ln, bias=half[:, 0:1], scale=0.5)
            o0t = po16.tile([128, NSHP], F16, tag="o16")
            for j in range(NSHP // 512):
                ps = ps3.tile([128, 512], F32, tag="mm")
                nc.tensor.matmul(ps[:], W_lastT,
                                 sv[:, 512 * j : 512 * (j + 1)],
                                 start=True, stop=True, skip_group_check=True)
                nc.vector.scalar_tensor_tensor(
                    o0t[:, 512 * j : 512 * (j + 1)], ps[:], b_last,
                    xuT[:, 512 * j : 512 * (j + 1)], ALU.add, ALU.add)
            o0r = prw3.tile([128, NBLK, 128], F16, tag="rw")
            nc.sync.dma_start_transpose(o0r[:], o0t[:, : NBLK * 128])
            nc.sync.dma_start(out0, o0r[:].rearrange("p a b -> p (a b)"))
    return nc


def kernel(**inputs):
    import concourse.bacc as bacc
    from concourse import bass_utils

    x = np.asarray(inputs["x"], np.float32)
    src_d, dstf_d, ea_d, meta = _prep(x, inputs["edge_index"], inputs["edge_attr"])

    key = (meta["S"], meta["G"])
    if key not in _cache:
        nc = bacc.Bacc("TRN2", target_bir_lowering=False, debug=False,
                       enable_asserts=False, num_devices=NC_,
                       dynamic_dma_scratch_size=32768)
        _build(nc, meta)
        nc.compile()
        _cache[key] = nc
    nc = _cache[key]

    wpack = np.concatenate(
        [np.asarray(inputs[k], np.float32).T.copy() for k in ["W_diff", "W_same"]]
        + [np.asarray(inputs["res_W1"][i], np.float32).T.copy() for i in range(3)]
        + [np.asarray(inputs["res_W2"][i], np.float32).T.copy() for i in range(3)]
        + [np.asarray(inputs["W_last"], np.float32).T.copy()], axis=1)
    bpack = np.zeros((F, 16), np.float32)
    bpack[:, 0] = np.asarray(inputs["b_diff"], np.float32)
    bpack[:, 1] = np.asarray(inputs["b_same"], np.float32)
    for i in range(3):
        bpack[:, 2 + i] = np.asarray(inputs["res_b1"][i], np.float32)
        bpack[:, 5 + i] = np.asarray(inputs["res_b2"][i], np.float32)
    bpack[:, 8] = np.asarray(inputs["b_last"], np.float32)
    bpack[:, 9] = np.asarray(inputs["u"], np.float32)[0]
    G_w = np.asarray(inputs["G_w"], np.float32)
    gw2 = np.zeros((128, 128), np.float16)
    gw2[:K] = G_w.T.astype(np.float16)
    gw2[64 : 64 + K] = G_w.T.astype(np.float16)
    iota = np.broadcast_to(np.arange(CHUNK, dtype=np.float16), (128, CHUNK)).copy()
    import ml_dtypes
    wdiffb = np.asarray(inputs["W_diff"], np.float32).T.copy().astype(ml_dtypes.bfloat16)
    wpackb = wpack.astype(ml_dtypes.bfloat16)

    xT = np.zeros((F, NPAD), np.float16)
    xT[:, :N] = x.T
    in_maps = []
    for c in range(NC_):
        xTo = np.zeros((F, NSHP), np.float16)
        xTo[:, :NSH] = x.T[:, c * NSH : (c + 1) * NSH]
        in_maps.append(dict(
            xT=xT, xTo=xTo, wpack=wpack, wpackb=wpackb, bpack=bpack, gw2=gw2, iota_in=iota,
            wdiffb_in=wdiffb, src_in=src_d[c], dstf_in=dstf_d[c], ea_in=ea_d[c],
        ))
    res = bass_utils.run_bass_kernel_spmd(nc, in_maps, core_ids=list(range(NC_)))

    def unblk(a):
        return (a.reshape(128, NBLK, 128).transpose(1, 0, 2)
                 .reshape(NBLK * 128, F)[:NSH].astype(np.float32))

    o0 = np.concatenate([unblk(res.results[c]["out0"]) for c in range(NC_)], axis=0)
    o1 = np.concatenate([unblk(res.results[c]["out1"]) for c in range(NC_)], axis=0)
    return (o0, o1)
